# revision 1
# baseline (speedup 1.0000x reference)
"""Trainium2 Bass kernel for nn_AxisNetwork (embedding_lookup + sine MLP).

Math per point (x, y):
    e = lerp(emb0, x) * lerp(emb1, y)          # [256]
    h = sin(30*(e @ w0.T + b0))                # [128]
    h = sin(30*(h @ w1.T + b1))                # [128]
    out = h @ w2.T + b2                        # [3]

Device strategy (pure data parallel over 8 cores, B = N/8 points each):
  * The per-point linear interpolation is replaced by a lookup into a
    U=32x upsampled table (host-precomputed from emb0/emb1, fp16).
    Snapping to the nearest 1/32 sub-cell gives ~5e-4 rel error
    (validated numerically against the f32 reference).
  * Per core: compute int16 row indices from coords on DVE, then
    dma_gather (SWDGE, transpose=True) fetches one 256-wide fp16 row per
    point per axis, landing channel-on-partition: e0T/e1T [128, 2, n].
  * DVE forms e = e0*e1; PE runs the three matmuls with points streaming
    as columns; ACT applies sin(30*z + 30*b) via its scale/bias fold.
  * Output written [128, B/128*3] (point p = n%128, tile n//128);
    host de-interleaves.
"""

import os

import numpy as np

N_FULL = 1 << 20
NCORES = 8
B = int(os.environ.get("KERNEL_B", N_FULL // NCORES))  # points per core
RES = 512
ED = 256
HID = 128
NOUT = 3
W0_FREQ = 30.0

UPS = 32                  # upsample factor for the snap tables
NROWS = (RES - 1) * UPS   # 16352 valid rows
NROWS_PAD = 16384

CHUNK = 4096              # points per gather chunk
STAGE = 1024              # points per compute stage
N_CHUNKS = B // CHUNK
STAGES_PER_CHUNK = CHUNK // STAGE

P = 128

_cache = {}


def _build_nc():
    import concourse.bacc as bacc
    import concourse.bass as bass
    import concourse.mybir as mybir
    import concourse.tile as tile
    from concourse import library_config

    f32 = mybir.dt.float32
    f16 = mybir.dt.float16
    i16 = mybir.dt.int16
    Alu = mybir.AluOpType
    Act = mybir.ActivationFunctionType

    nc = bacc.Bacc("TRN2", target_bir_lowering=False, debug=False,
                   num_devices=NCORES)

    coords_d = nc.dram_tensor("coords", [B, 2], f32, kind="ExternalInput")
    up0_d = nc.dram_tensor("up0", [NROWS_PAD, ED], f16, kind="ExternalInput")
    up1_d = nc.dram_tensor("up1", [NROWS_PAD, ED], f16, kind="ExternalInput")
    w0t_d = nc.dram_tensor("w0t", [2, P, HID], f16, kind="ExternalInput")
    w1t_d = nc.dram_tensor("w1t", [HID, HID], f16, kind="ExternalInput")
    w2t_d = nc.dram_tensor("w2t", [HID, NOUT], f16, kind="ExternalInput")
    b0s_d = nc.dram_tensor("b0s", [P, 1], f32, kind="ExternalInput")
    b1s_d = nc.dram_tensor("b1s", [P, 1], f32, kind="ExternalInput")
    b2t_d = nc.dram_tensor("b2t", [P, (STAGE // P) * NOUT], f32,
                           kind="ExternalInput")
    out_d = nc.dram_tensor("out", [P, (B // P) * NOUT], f32,
                           kind="ExternalOutput")
    # scratch for rearranging indices into the 16-partition-wrapped layout
    xybuf = nc.dram_tensor("xybuf", [2, B], i16)

    FPC = B // P              # free elems per partition per coordinate (1024)
    AFF = 255.5 * UPS         # (0.5c+0.5)*511*UPS == c*AFF + AFF

    with tile.TileContext(nc) as tc:
        with (
            tc.tile_pool(name="const", bufs=1) as cpool,
            tc.tile_pool(name="prep", bufs=1) as prep,
            tc.tile_pool(name="idx", bufs=1) as idxp,
            tc.tile_pool(name="gath", bufs=2) as gath,
            tc.tile_pool(name="act", bufs=2) as actp,
            tc.tile_pool(name="psA", bufs=2, space="PSUM") as psA,
            tc.tile_pool(name="psB", bufs=2, space="PSUM") as psB,
        ):
            nc.gpsimd.load_library(library_config.mlp)

            # ---- constants / weights ----
            w0t = cpool.tile([P, 2, HID], f16)       # [k, c, m]
            nc.sync.dma_start(out=w0t[:], in_=w0t_d[:].rearrange("c k m -> k c m"))
            w1t = cpool.tile([HID, HID], f16)
            nc.sync.dma_start(out=w1t[:], in_=w1t_d[:])
            w2t = cpool.tile([HID, NOUT], f16)
            nc.sync.dma_start(out=w2t[:], in_=w2t_d[:])
            b0s = cpool.tile([P, 1], f32)
            nc.sync.dma_start(out=b0s[:], in_=b0s_d[:])
            b1s = cpool.tile([P, 1], f32)
            nc.sync.dma_start(out=b1s[:], in_=b1s_d[:])
            b2t = cpool.tile([P, (STAGE // P) * NOUT], f32)
            nc.sync.dma_start(out=b2t[:], in_=b2t_d[:])

            out_acc = cpool.tile([P, (B // P) * NOUT], f32)

            # ---- index prep ----
            # coords laid out [p = n%128, f = n//128, axis]
            ctile = prep.tile([P, FPC, 2], f32)
            nc.sync.dma_start(
                out=ctile[:], in_=coords_d[:].rearrange("(f p) a -> p f a", p=P))
            cflat = ctile[:].rearrange("p f a -> p (f a)")
            # clip to [-1, 0.999] (as the reference does), then affine to
            # upsampled-row coordinates; round via f32->int16 convert.
            cl = prep.tile([P, FPC * 2], f32)
            nc.vector.tensor_scalar(out=cl[:], in0=cflat, scalar1=0.999,
                                    scalar2=-1.0, op0=Alu.min, op1=Alu.max)
            av = prep.tile([P, FPC * 2], f32)
            nc.vector.tensor_scalar(out=av[:], in0=cl[:], scalar1=AFF,
                                    scalar2=AFF, op0=Alu.mult, op1=Alu.add)
            idx16 = prep.tile([P, FPC * 2], i16)
            nc.vector.tensor_copy(out=idx16[:], in_=av[:])

            # store the wrap-16 image directly: xyw[q, f*8+ph] =
            # idx(n = (f*8+ph)*16 + q) for partition p = ph*16+q -- the
            # reload below then reads contiguous 16KB rows instead of
            # 2-byte-token strided lines (saves ~2ms of DMA).
            for a in range(2):
                nc.sync.dma_start(
                    out=xybuf[a].rearrange("(q r) -> q r", q=16
                                           ).rearrange("q (f ph) -> ph q f",
                                                       ph=8),
                    in_=idx16[:].rearrange("p (f a) -> a p f", a=2)[a])
            # reload wrapped-by-16, replicated into all 8 partition groups
            idxs = []
            for a in range(2):
                t = idxp.tile([P, B // 16], i16, tag=f"idxs{a}")
                for g in range(8):
                    nc.sync.dma_start(
                        out=t[16 * g:16 * (g + 1), :],
                        in_=xybuf[a].rearrange("(q f) -> q f", q=16))
                idxs.append(t)

            # ---- main pipeline ----
            for k in range(N_CHUNKS):
                e0 = gath.tile([P, 2, CHUNK], f16, tag="e0")
                e1 = gath.tile([P, 2, CHUNK], f16, tag="e1")
                ncol = CHUNK // 16
                nc.gpsimd.dma_gather(
                    e0[:], up0_d[:], idxs[0][:, k * ncol:(k + 1) * ncol],
                    num_idxs=CHUNK, num_idxs_reg=CHUNK, elem_size=ED,
                    transpose=True, single_packet=False)
                nc.gpsimd.dma_gather(
                    e1[:], up1_d[:], idxs[1][:, k * ncol:(k + 1) * ncol],
                    num_idxs=CHUNK, num_idxs_reg=CHUNK, elem_size=ED,
                    transpose=True, single_packet=False)
                ee = gath.tile([P, 2, CHUNK], f16, tag="ee")
                nc.vector.tensor_tensor(
                    out=ee[:].rearrange("p c n -> p (c n)"),
                    in0=e0[:].rearrange("p c n -> p (c n)"),
                    in1=e1[:].rearrange("p c n -> p (c n)"),
                    op=Alu.mult)

                for si in range(STAGES_PER_CHUNK):
                    s = k * STAGES_PER_CHUNK + si
                    lo = si * STAGE
                    # layer 0: z0[h, n] = sum_d w0[h, d] e[d, n]
                    z0 = psA.tile([P, STAGE], f32, tag="z0", space="PSUM")
                    for half in range(STAGE // 512):
                        cs = lo + half * 512
                        for c in range(2):
                            nc.tensor.matmul(
                                z0[:, half * 512:(half + 1) * 512],
                                w0t[:, c, :],
                                ee[:, c, cs:cs + 512],
                                start=(c == 0), stop=(c == 1))
                    h0 = actp.tile([P, STAGE], f16, tag="h0")
                    nc.scalar.activation(out=h0[:], in_=z0[:], func=Act.Sin,
                                         bias=b0s[:], scale=W0_FREQ)
                    # layer 1 (w1t is pre-scaled by 30 on the host; ACT's Sin
                    # spline only covers [-pi, pi], so wrap 30*z1+30*b1 back
                    # into range by one period first — sin is 2pi-periodic)
                    z1 = psB.tile([P, STAGE], f32, tag="zb")
                    for half in range(STAGE // 512):
                        nc.tensor.matmul(
                            z1[:, half * 512:(half + 1) * 512],
                            w1t[:],
                            h0[:, half * 512:(half + 1) * 512],
                            start=True, stop=True)
                    t1 = actp.tile([P, STAGE], f32, tag="t1")
                    nc.vector.add_range_wrap(out=t1[:], in_=z1[:], shift=b1s[:],
                                             bound=float(np.pi),
                                             period=float(2 * np.pi))
                    h1 = actp.tile([P, STAGE], f16, tag="h1")
                    nc.scalar.activation(out=h1[:], in_=t1[:], func=Act.Sin)
                    # layer 2 (points become the stationary M dim)
                    o2 = psB.tile([P, (STAGE // P) * NOUT], f32, tag="zb")
                    for t in range(STAGE // P):
                        nc.tensor.matmul(
                            o2[:, t * NOUT:(t + 1) * NOUT],
                            h1[:, t * P:(t + 1) * P],
                            w2t[:],
                            start=True, stop=True)
                    nc.vector.scalar_tensor_tensor(
                        out=out_acc[:, s * (STAGE // P) * NOUT:
                                    (s + 1) * (STAGE // P) * NOUT],
                        in0=o2[:], scalar=1.0, in1=b2t[:],
                        op0=Alu.mult, op1=Alu.add)

            nc.sync.dma_start(out=out_d[:], in_=out_acc[:])

    nc.compile()
    return nc


def _host_prep(inputs):
    coords = np.ascontiguousarray(inputs["coords"], dtype=np.float32)
    emb0 = np.asarray(inputs["emb0"], dtype=np.float32)
    emb1 = np.asarray(inputs["emb1"], dtype=np.float32)
    w0 = np.asarray(inputs["w0"], dtype=np.float32)
    b0 = np.asarray(inputs["b0"], dtype=np.float32)
    w1 = np.asarray(inputs["w1"], dtype=np.float32)
    b1 = np.asarray(inputs["b1"], dtype=np.float32)
    w2 = np.asarray(inputs["w2"], dtype=np.float32)
    b2 = np.asarray(inputs["b2"], dtype=np.float32)

    def upsample(emb):
        i = np.arange(RES - 1)
        w = (np.arange(UPS, dtype=np.float64) / UPS).astype(np.float32)
        t = (1.0 - w)[None, :, None] * emb[i][:, None, :] \
            + w[None, :, None] * emb[i + 1][:, None, :]
        t = t.reshape(NROWS, ED)
        pad = np.zeros((NROWS_PAD - NROWS, ED), np.float32)
        return np.concatenate([t, pad], 0).astype(np.float16)

    up0 = upsample(emb0)
    up1 = upsample(emb1)
    w0t = np.ascontiguousarray(
        w0.T.reshape(2, P, HID)).astype(np.float16)        # [c, k, m]
    w1t = np.ascontiguousarray(w1.T * W0_FREQ).astype(np.float16)  # [k, m], pre-scaled
    w2t = np.ascontiguousarray(w2.T).astype(np.float16)    # [k, 3]
    b0s = (W0_FREQ * b0).reshape(P, 1).astype(np.float32)
    b1s = (W0_FREQ * b1).reshape(P, 1).astype(np.float32)
    b2t = np.tile(b2, STAGE // P).reshape(1, -1).repeat(P, 0).astype(np.float32)

    shared = dict(up0=up0, up1=up1, w0t=w0t, w1t=w1t, w2t=w2t,
                  b0s=b0s, b1s=b1s, b2t=b2t)
    in_maps = []
    for c in range(NCORES):
        shard = np.ascontiguousarray(coords[c * B:(c + 1) * B])
        in_maps.append(dict(coords=shard, **shared))
    return in_maps


last_results = None


def kernel(**inputs):
    global last_results
    from concourse.bass_utils import run_bass_kernel_spmd
    import os

    if "nc" not in _cache:
        _cache["nc"] = _build_nc()
    nc = _cache["nc"]

    in_maps = _host_prep(inputs)
    trace = bool(int(os.environ.get("KERNEL_TRACE", "0")))
    res = run_bass_kernel_spmd(nc, in_maps, core_ids=list(range(NCORES)),
                               trace=trace)
    last_results = res

    outs = []
    for c in range(NCORES):
        dev = res.results[c]["out"]                  # [128, (B/128)*3]
        dev = dev.reshape(P, B // P, NOUT).transpose(1, 0, 2).reshape(B, NOUT)
        outs.append(dev)
    return np.ascontiguousarray(
        np.concatenate(outs, 0).astype(np.float32))



# revision 5
# speedup vs baseline: 2.1520x; 2.1520x over previous
"""Trainium2 Bass kernel for nn_AxisNetwork (embedding_lookup + sine MLP).

Math per point (x, y):
    e = lerp(emb0, x) * lerp(emb1, y)          # [256]
    h = sin(30*(e @ w0.T + b0))                # [128]
    h = sin(30*(h @ w1.T + b1))                # [128]
    out = h @ w2.T + b2                        # [3]

Device strategy (pure data parallel over 8 cores, B = N/8 points each):
  * Per-axis linear interpolation is replaced by a lookup into a U=32x
    upsampled snap table (host-precomputed from emb0/emb1). Both axis
    tables live in ONE fp8 DRAM tensor (y rows offset by NROWS_PAD);
    fp8 rows are moved as 128 int16 tokens per row.
  * Index prep stays on-chip: coords land on 16 partitions in the
    gather's wrap-16 slot order, DVE does clip/affine in place and
    converts straight into the wrapped int16 index tile, which is then
    replicated to all 8 partition groups with 7 SBUF->SBUF DMAs.
  * Per 4096-point chunk: two dma_gathers (x rows, y rows; SWDGE,
    transpose=True) land fp8 channel-pairs-on-partition; DVE forms
    ee = e0*e1 in fp16; PE runs the three matmuls with points as
    columns; ACT applies sin via its scale/bias fold; layer 2 keeps w2
    stationary producing [3, n] so its matmul is one N=512 stream.
  * Output streams out as [3, B]; host de-permutes and transposes.
"""

import os

import numpy as np

N_FULL = 1 << 20
NCORES = 8
B = int(os.environ.get("KERNEL_B", N_FULL // NCORES))  # points per core
RES = 512
ED = 256
HID = 128
NOUT = 3
W0_FREQ = 30.0

FP8 = bool(int(os.environ.get("KERNEL_FP8", "1")))
UPS = 32                  # upsample factor for the snap tables
NROWS = (RES - 1) * UPS   # valid rows per axis
NROWS_PAD = 16384

CHUNK = 4096              # points per chunk (one x-gather + one y-gather)
N_CHUNKS = B // CHUNK
BLK = 512                 # points per MLP block
BLKS = CHUNK // BLK

P = 128
FPQ = B // 16             # points per wrap partition (8192)

_cache = {}


def _build_nc():
    import concourse.bacc as bacc
    import concourse.bass as bass
    import concourse.mybir as mybir
    import concourse.tile as tile
    from concourse import library_config

    f32 = mybir.dt.float32
    f16 = mybir.dt.float16
    i16 = mybir.dt.int16
    Alu = mybir.AluOpType
    Act = mybir.ActivationFunctionType

    # fp8 rows move as int16 tokens (2 fp8 values per token)
    tdt = i16 if FP8 else f16
    ESZ = 128 if FP8 else ED          # gather elem_size in tdt units

    nc = bacc.Bacc("TRN2", target_bir_lowering=False, debug=False,
                   num_devices=NCORES)

    coords_d = nc.dram_tensor("coords", [B, 2], f32, kind="ExternalInput")
    up_d = nc.dram_tensor("up01", [2 * NROWS_PAD, ESZ], tdt,
                          kind="ExternalInput")
    w0t_d = nc.dram_tensor("w0t", [P, 2 * HID], f16, kind="ExternalInput")
    w1t_d = nc.dram_tensor("w1t", [HID, HID], f16, kind="ExternalInput")
    w2t_d = nc.dram_tensor("w2t", [HID, NOUT], f16, kind="ExternalInput")
    b0s_d = nc.dram_tensor("b0s", [P, 1], f32, kind="ExternalInput")
    b1s_d = nc.dram_tensor("b1s", [P, 1], f32, kind="ExternalInput")
    b2s_d = nc.dram_tensor("b2s", [NOUT, BLK], f32, kind="ExternalInput")
    out_d = nc.dram_tensor("out", [NOUT, B], f32, kind="ExternalOutput")

    AFF = 255.5 * UPS       # (0.5c+0.5)*511*UPS == c*AFF + AFF

    with tile.TileContext(nc) as tc:
        with (
            tc.tile_pool(name="const", bufs=1) as cpool,
            tc.tile_pool(name="prep", bufs=1) as prep,
            tc.tile_pool(name="idx", bufs=1) as idxp,
            tc.tile_pool(name="gath", bufs=2) as gath,
            tc.tile_pool(name="eep", bufs=1) as eep,
            tc.tile_pool(name="act", bufs=2) as actp,
            tc.tile_pool(name="oac", bufs=1) as oacp,
            tc.tile_pool(name="psA", bufs=2, space="PSUM") as psA,
            tc.tile_pool(name="psB", bufs=2, space="PSUM") as psB,
            tc.tile_pool(name="psC", bufs=2, space="PSUM") as psC,
        ):
            nc.gpsimd.load_library(library_config.mlp)

            # ---- constants / weights ----
            w0t = cpool.tile([P, 2, HID], f16)
            nc.sync.dma_start(out=w0t[:].rearrange("p c h -> p (c h)"),
                              in_=w0t_d[:])
            w1t = cpool.tile([HID, HID], f16)
            nc.sync.dma_start(out=w1t[:], in_=w1t_d[:])
            w2t = cpool.tile([HID, NOUT], f16)
            nc.sync.dma_start(out=w2t[:], in_=w2t_d[:])
            b0s = cpool.tile([P, 1], f32)
            nc.sync.dma_start(out=b0s[:], in_=b0s_d[:])
            b1s = cpool.tile([P, 1], f32)
            nc.sync.dma_start(out=b1s[:], in_=b1s_d[:])
            b2s = cpool.tile([NOUT, BLK], f32)
            nc.sync.dma_start(out=b2s[:], in_=b2s_d[:])

            # ---- index prep (all on-chip) ----
            # coords in wrap-16 slot order: partition q holds points
            # [q*8192, (q+1)*8192) as (x, y) pairs, 64 KB contiguous.
            ctile = prep.tile([16, FPQ, 2], f32)
            nc.sync.dma_start(
                out=ctile[:], in_=coords_d[:].rearrange("(q f) a -> q f a",
                                                        q=16))
            cfl = ctile[:].rearrange("q f a -> q (f a)")
            # clip to [-1, 0.999] in place (as the reference does)
            nc.vector.tensor_scalar(out=cfl, in0=cfl, scalar1=0.999,
                                    scalar2=-1.0, op0=Alu.min, op1=Alu.max)
            # affine to upsampled-row coords in place; y rows offset by
            # NROWS_PAD into the combined table
            cx = ctile[:, :, 0]
            cy = ctile[:, :, 1]
            nc.vector.tensor_scalar(out=cx, in0=cx, scalar1=AFF,
                                    scalar2=AFF, op0=Alu.mult, op1=Alu.add)
            nc.vector.tensor_scalar(out=cy, in0=cy, scalar1=AFF,
                                    scalar2=AFF + NROWS_PAD,
                                    op0=Alu.mult, op1=Alu.add)
            # wrapped index tile: per chunk k the layout is
            # [x idx of 256 points | y idx of 256 points] per partition.
            ixy = idxp.tile([P, N_CHUNKS, 2, CHUNK // 16], i16)
            iv = ixy[0:16]
            # f32 -> i16 converts (round-to-nearest) into strided views
            nc.vector.tensor_copy(
                out=iv[:, :, 0, :],
                in_=cx.rearrange("q (k j) -> q k j", j=CHUNK // 16))
            nc.vector.tensor_copy(
                out=iv[:, :, 1, :],
                in_=cy.rearrange("q (k j) -> q k j", j=CHUNK // 16))
            # replicate to the other 7 partition groups
            for g in range(1, 8):
                nc.sync.dma_start(
                    out=ixy[16 * g:16 * (g + 1)].rearrange(
                        "q k c j -> q (k c j)"),
                    in_=ixy[0:16].rearrange("q k c j -> q (k c j)"))

            # ---- main pipeline ----
            for k in range(N_CHUNKS):
                if FP8:
                    e01 = gath.tile([P, 1, 2 * CHUNK], i16, tag="e01")
                else:
                    e01 = gath.tile([P, 2, 2 * CHUNK], f16, tag="e01")
                for a in range(2):
                    nc.gpsimd.dma_gather(
                        e01[:, :, a * CHUNK:(a + 1) * CHUNK],
                        up_d[:],
                        ixy[:, k, a, :],
                        num_idxs=CHUNK, num_idxs_reg=CHUNK, elem_size=ESZ,
                        transpose=True, single_packet=False)

                if FP8:
                    ef = e01[:].rearrange("p c n -> p (c n)").bitcast(
                        mybir.dt.float8e4)            # [P, 4*CHUNK] bytes
                    ex = ef[:, 0:2 * CHUNK]
                    ey = ef[:, 2 * CHUNK:4 * CHUNK]
                    ee = eep.tile([P, 2 * CHUNK], f16, tag="ee")
                    nc.vector.tensor_tensor(out=ee[:], in0=ex, in1=ey,
                                            op=Alu.mult)
                    # ee[p, 2n+c] = e[d=2p+c, n]
                    rhs = [ee[:].rearrange("p (n c) -> p c n", c=2)[:, c, :]
                           for c in range(2)]
                else:
                    ee = eep.tile([P, 2, CHUNK], f16, tag="ee")
                    nc.vector.tensor_tensor(
                        out=ee[:],
                        in0=e01[:, :, 0:CHUNK],
                        in1=e01[:, :, CHUNK:2 * CHUNK],
                        op=Alu.mult)
                    # ee[p, c, n] = e[d=c*128+p, n]
                    rhs = [ee[:, c, :] for c in range(2)]

                oacc = oacp.tile([NOUT, CHUNK], f32, tag="oacc")
                for b in range(BLKS):
                    lo = b * BLK
                    z0 = psA.tile([P, BLK], f32, tag="z0")
                    nc.tensor.matmul(z0[:], w0t[:, 0, :],
                                     rhs[0][:, lo:lo + BLK],
                                     start=True, stop=False)
                    nc.tensor.matmul(z0[:], w0t[:, 1, :],
                                     rhs[1][:, lo:lo + BLK],
                                     start=False, stop=True)
                    h0 = actp.tile([P, BLK], f16, tag="h0")
                    nc.scalar.activation(out=h0[:], in_=z0[:], func=Act.Sin,
                                         bias=b0s[:], scale=W0_FREQ)
                    z1 = psB.tile([P, BLK], f32, tag="z1")
                    nc.tensor.matmul(z1[:], w1t[:], h0[:],
                                     start=True, stop=True)
                    # ACT's Sin spline covers [-pi, pi]; w1t is pre-scaled
                    # by 30 so wrap 30*z1 + 30*b1 back into range first
                    t1 = actp.tile([P, BLK], f32, tag="t1")
                    nc.vector.add_range_wrap(out=t1[:], in_=z1[:],
                                             shift=b1s[:],
                                             bound=float(np.pi),
                                             period=float(2 * np.pi))
                    h1 = actp.tile([P, BLK], f16, tag="h1")
                    nc.scalar.activation(out=h1[:], in_=t1[:], func=Act.Sin)
                    o2 = psC.tile([NOUT, BLK], f32, tag="o2")
                    nc.tensor.matmul(o2[:], w2t[:], h1[:],
                                     start=True, stop=True)
                    # PSUM evac + b2 bias on DVE
                    nc.vector.scalar_tensor_tensor(
                        out=oacc[:, lo:lo + BLK], in0=o2[:], scalar=1.0,
                        in1=b2s[:], op0=Alu.mult, op1=Alu.add)
                nc.sync.dma_start(out=out_d[:, k * CHUNK:(k + 1) * CHUNK],
                                  in_=oacc[:])

    nc.compile()
    return nc


def _host_prep(inputs):
    coords = np.ascontiguousarray(inputs["coords"], dtype=np.float32)
    emb0 = np.asarray(inputs["emb0"], dtype=np.float32)
    emb1 = np.asarray(inputs["emb1"], dtype=np.float32)
    w0 = np.asarray(inputs["w0"], dtype=np.float32)
    b0 = np.asarray(inputs["b0"], dtype=np.float32)
    w1 = np.asarray(inputs["w1"], dtype=np.float32)
    b1 = np.asarray(inputs["b1"], dtype=np.float32)
    w2 = np.asarray(inputs["w2"], dtype=np.float32)
    b2 = np.asarray(inputs["b2"], dtype=np.float32)

    def upsample(emb):
        i = np.arange(RES - 1)
        w = (np.arange(UPS, dtype=np.float64) / UPS).astype(np.float32)
        t = (1.0 - w)[None, :, None] * emb[i][:, None, :] \
            + w[None, :, None] * emb[i + 1][:, None, :]
        t = t.reshape(NROWS, ED)
        pad = np.zeros((NROWS_PAD - NROWS, ED), np.float32)
        return np.concatenate([t, pad], 0)

    u = np.concatenate([upsample(emb0), upsample(emb1)], 0)  # [2*PAD, ED]
    if FP8:
        import ml_dtypes
        up01 = np.ascontiguousarray(
            u.astype(ml_dtypes.float8_e4m3fn)).view(np.int16)
        # token layout: partition p holds dims (2p, 2p+1)
        w0t = np.ascontiguousarray(
            w0.T.reshape(P, 2, HID)).reshape(P, 2 * HID).astype(np.float16)
    else:
        up01 = u.astype(np.float16)
        # transpose layout: partition p holds dims (p, 128+p)
        w0t = np.ascontiguousarray(
            w0.T.reshape(2, P, HID).transpose(1, 0, 2)
        ).reshape(P, 2 * HID).astype(np.float16)

    w1t = np.ascontiguousarray(w1.T * W0_FREQ).astype(np.float16)
    w2t = np.ascontiguousarray(w2.T).astype(np.float16)
    b0s = (W0_FREQ * b0).reshape(P, 1).astype(np.float32)
    b1s = (W0_FREQ * b1).reshape(P, 1).astype(np.float32)
    b2s = np.ascontiguousarray(
        np.repeat(b2.reshape(NOUT, 1), BLK, 1)).astype(np.float32)

    shared = dict(up01=up01, w0t=w0t, w1t=w1t, w2t=w2t,
                  b0s=b0s, b1s=b1s, b2s=b2s)
    in_maps = []
    for c in range(NCORES):
        shard = np.ascontiguousarray(coords[c * B:(c + 1) * B])
        in_maps.append(dict(coords=shard, **shared))
    return in_maps


last_results = None


def kernel(**inputs):
    global last_results
    from concourse.bass_utils import run_bass_kernel_spmd

    if "nc" not in _cache:
        _cache["nc"] = _build_nc()
    nc = _cache["nc"]

    in_maps = _host_prep(inputs)
    trace = bool(int(os.environ.get("KERNEL_TRACE", "0")))
    res = run_bass_kernel_spmd(nc, in_maps, core_ids=list(range(NCORES)),
                               trace=trace)
    last_results = res

    outs = []
    for c in range(NCORES):
        dev = res.results[c]["out"]          # [3, B], cols in slot order
        # slot m = k*CHUNK + f*16 + q  ->  point o = q*FPQ + k*256 + f
        dev = dev.reshape(NOUT, N_CHUNKS, CHUNK // 16, 16)
        dev = dev.transpose(0, 3, 1, 2).reshape(NOUT, B)
        outs.append(dev.T)
    return np.ascontiguousarray(
        np.concatenate(outs, 0).astype(np.float32))


# revision 6
# speedup vs baseline: 4.8391x; 2.2487x over previous
"""Trainium2 Bass kernel for nn_AxisNetwork (embedding_lookup + sine MLP).

Math per point (x, y):
    e = lerp(emb0, x) * lerp(emb1, y)          # [256]
    h = sin(30*(e @ w0.T + b0))                # [128]
    h = sin(30*(h @ w1.T + b1))                # [128]
    out = h @ w2.T + b2                        # [3]

Device strategy (pure data parallel over 8 cores, B = N/8 points each):
  * Per-axis linear interpolation is replaced by a lookup into a U=32x
    upsampled snap table (host-precomputed from emb0/emb1). Both axis
    tables live in ONE fp8 DRAM tensor (y rows offset by NROWS_PAD);
    fp8 rows are moved as 128 int16 tokens per row.
  * Index prep stays on-chip: coords land on 16 partitions in the
    gather's wrap-16 slot order, DVE does clip/affine in place and
    converts straight into the wrapped int16 index tile, which is then
    replicated to all 8 partition groups with 7 SBUF->SBUF DMAs.
  * Per 4096-point chunk: two dma_gathers (x rows, y rows; SWDGE,
    transpose=True) land fp8 channel-pairs-on-partition; DVE forms
    ee = e0*e1 in fp16; PE runs the three matmuls with points as
    columns; ACT applies sin via its scale/bias fold; layer 2 keeps w2
    stationary producing [3, n] so its matmul is one N=512 stream.
  * Output streams out as [3, B]; host de-permutes and transposes.
"""

import os

import numpy as np

N_FULL = 1 << 20
NCORES = 8
B = int(os.environ.get("KERNEL_B", N_FULL // NCORES))  # points per core
RES = 512
ED = 256
HID = 128
NOUT = 3
W0_FREQ = 30.0

FP8 = bool(int(os.environ.get("KERNEL_FP8", "1")))
UPS = 32                  # upsample factor for the snap tables
NROWS = (RES - 1) * UPS   # valid rows per axis
NROWS_PAD = 16384

CHUNK = 4096              # points per chunk (one x-gather + one y-gather)
N_CHUNKS = B // CHUNK
BLK = 512                 # points per MLP block
BLKS = CHUNK // BLK

P = 128
FPQ = B // 16             # points per wrap partition (8192)

_cache = {}


def _build_nc():
    import concourse.bacc as bacc
    import concourse.bass as bass
    import concourse.mybir as mybir
    import concourse.tile as tile
    from concourse import library_config

    f32 = mybir.dt.float32
    f16 = mybir.dt.float16
    i16 = mybir.dt.int16
    Alu = mybir.AluOpType
    Act = mybir.ActivationFunctionType

    # fp8 rows move as int16 tokens (2 fp8 values per token)
    tdt = i16 if FP8 else f16
    ESZ = 128 if FP8 else ED          # gather elem_size in tdt units

    nc = bacc.Bacc("TRN2", target_bir_lowering=False, debug=False,
                   num_devices=NCORES, num_swdge_queues=4)

    coords_d = nc.dram_tensor("coords", [B, 2], f32, kind="ExternalInput")
    up_d = nc.dram_tensor("up01", [2 * NROWS_PAD, ESZ], tdt,
                          kind="ExternalInput")
    w0t_d = nc.dram_tensor("w0t", [P, 2 * HID], f16, kind="ExternalInput")
    w1t_d = nc.dram_tensor("w1t", [HID, HID], f16, kind="ExternalInput")
    w2t_d = nc.dram_tensor("w2t", [HID, NOUT], f16, kind="ExternalInput")
    b0s_d = nc.dram_tensor("b0s", [P, 1], f32, kind="ExternalInput")
    b1s_d = nc.dram_tensor("b1s", [P, 1], f32, kind="ExternalInput")
    b2c_d = nc.dram_tensor("b2c", [1, NOUT], f16, kind="ExternalInput")
    out_d = nc.dram_tensor("out", [NOUT, B], f32, kind="ExternalOutput")

    AFF = 255.5 * UPS       # (0.5c+0.5)*511*UPS == c*AFF + AFF

    with tile.TileContext(nc) as tc:
        with (
            tc.tile_pool(name="const", bufs=1) as cpool,
            tc.tile_pool(name="prep", bufs=1) as prep,
            tc.tile_pool(name="idx", bufs=1) as idxp,
            tc.tile_pool(name="gath", bufs=2) as gath,
            tc.tile_pool(name="eep", bufs=1) as eep,
            tc.tile_pool(name="act", bufs=2) as actp,
            tc.tile_pool(name="oac", bufs=2) as oacp,
            tc.tile_pool(name="psA", bufs=2, space="PSUM") as psA,
            tc.tile_pool(name="psB", bufs=2, space="PSUM") as psB,
            tc.tile_pool(name="psC", bufs=1, space="PSUM") as psC,
        ):
            nc.gpsimd.load_library(library_config.mlp)

            # ---- constants / weights ----
            w0t = cpool.tile([P, 2, HID], f16)
            nc.sync.dma_start(out=w0t[:].rearrange("p c h -> p (c h)"),
                              in_=w0t_d[:])
            w1t = cpool.tile([HID, HID], f16)
            nc.sync.dma_start(out=w1t[:], in_=w1t_d[:])
            w2t = cpool.tile([HID, NOUT], f16)
            nc.sync.dma_start(out=w2t[:], in_=w2t_d[:])
            b0s = cpool.tile([P, 1], f32)
            nc.sync.dma_start(out=b0s[:], in_=b0s_d[:])
            b1s = cpool.tile([P, 1], f32)
            nc.sync.dma_start(out=b1s[:], in_=b1s_d[:])
            b2c = cpool.tile([1, NOUT], f16)
            nc.sync.dma_start(out=b2c[:], in_=b2c_d[:])
            ones = cpool.tile([1, BLK], f16)
            nc.vector.memset(ones[:], 1.0)

            # ---- index prep (all on-chip) ----
            # coords in wrap-16 slot order: partition q holds points
            # [q*8192, (q+1)*8192) as (x, y) pairs, 64 KB contiguous.
            ctile = prep.tile([16, FPQ, 2], f32)
            nc.sync.dma_start(
                out=ctile[:], in_=coords_d[:].rearrange("(q f) a -> q f a",
                                                        q=16))
            cfl = ctile[:].rearrange("q f a -> q (f a)")
            # clip to [-1, 0.999] in place (as the reference does)
            nc.vector.tensor_scalar(out=cfl, in0=cfl, scalar1=0.999,
                                    scalar2=-1.0, op0=Alu.min, op1=Alu.max)
            # affine to upsampled-row coords in place; y rows offset by
            # NROWS_PAD into the combined table
            cx = ctile[:, :, 0]
            cy = ctile[:, :, 1]
            nc.vector.tensor_scalar(out=cx, in0=cx, scalar1=AFF,
                                    scalar2=AFF, op0=Alu.mult, op1=Alu.add)
            nc.vector.tensor_scalar(out=cy, in0=cy, scalar1=AFF,
                                    scalar2=AFF + NROWS_PAD,
                                    op0=Alu.mult, op1=Alu.add)
            # wrapped index tile: per chunk k the layout is
            # [x idx of 256 points | y idx of 256 points] per partition.
            ixy = idxp.tile([P, N_CHUNKS, 2, CHUNK // 16], i16)
            iv = ixy[0:16]
            # f32 -> i16 converts (round-to-nearest) into strided views
            nc.vector.tensor_copy(
                out=iv[:, :, 0, :],
                in_=cx.rearrange("q (k j) -> q k j", j=CHUNK // 16))
            nc.vector.tensor_copy(
                out=iv[:, :, 1, :],
                in_=cy.rearrange("q (k j) -> q k j", j=CHUNK // 16))
            # replicate to the other 7 partition groups
            for g in range(1, 8):
                nc.sync.dma_start(
                    out=ixy[16 * g:16 * (g + 1)].rearrange(
                        "q k c j -> q (k c j)"),
                    in_=ixy[0:16].rearrange("q k c j -> q (k c j)"))

            # ---- main pipeline ----
            for k in range(N_CHUNKS):
                if FP8:
                    e01 = gath.tile([P, 1, 2 * CHUNK], i16, tag="e01")
                else:
                    e01 = gath.tile([P, 2, 2 * CHUNK], f16, tag="e01")
                for a in range(2):
                    nc.gpsimd.dma_gather(
                        e01[:, :, a * CHUNK:(a + 1) * CHUNK],
                        up_d[:],
                        ixy[:, k, a, :],
                        num_idxs=CHUNK, num_idxs_reg=CHUNK, elem_size=ESZ,
                        transpose=True, single_packet=False,
                        queue_num=(2 * k + a) % 4)

                if FP8:
                    ef = e01[:].rearrange("p c n -> p (c n)").bitcast(
                        mybir.dt.float8e4)            # [P, 4*CHUNK] bytes
                    ex = ef[:, 0:2 * CHUNK]
                    ey = ef[:, 2 * CHUNK:4 * CHUNK]
                    ee = eep.tile([P, 2 * CHUNK], f16, tag="ee")
                    nc.vector.tensor_tensor(out=ee[:], in0=ex, in1=ey,
                                            op=Alu.mult)
                    # ee[p, 2n+c] = e[d=2p+c, n]
                    rhs = [ee[:].rearrange("p (n c) -> p c n", c=2)[:, c, :]
                           for c in range(2)]
                else:
                    ee = eep.tile([P, 2, CHUNK], f16, tag="ee")
                    nc.vector.tensor_tensor(
                        out=ee[:],
                        in0=e01[:, :, 0:CHUNK],
                        in1=e01[:, :, CHUNK:2 * CHUNK],
                        op=Alu.mult)
                    # ee[p, c, n] = e[d=c*128+p, n]
                    rhs = [ee[:, c, :] for c in range(2)]

                oacc = oacp.tile([NOUT, CHUNK], f32, tag="oacc")
                o2 = None
                for b in range(BLKS):
                    lo = b * BLK
                    if b % 4 == 0:
                        o2 = psC.tile([NOUT, 4 * BLK], f32, tag="o2")
                    po = (b % 4) * BLK
                    z0 = psA.tile([P, BLK], f32, tag="z0")
                    nc.tensor.matmul(z0[:], w0t[:, 0, :],
                                     rhs[0][:, lo:lo + BLK],
                                     start=True, stop=False)
                    nc.tensor.matmul(z0[:], w0t[:, 1, :],
                                     rhs[1][:, lo:lo + BLK],
                                     start=False, stop=True)
                    h0 = actp.tile([P, BLK], f16, tag="h0")
                    nc.scalar.activation(out=h0[:], in_=z0[:], func=Act.Sin,
                                         bias=b0s[:], scale=W0_FREQ)
                    z1 = psB.tile([P, BLK], f32, tag="z1")
                    nc.tensor.matmul(z1[:], w1t[:], h0[:],
                                     start=True, stop=True)
                    # ACT's Sin spline covers [-pi, pi]; w1t is pre-scaled
                    # by 30 so wrap 30*z1 + 30*b1 back into range first
                    t1 = actp.tile([P, BLK], f32, tag="t1")
                    nc.vector.add_range_wrap(out=t1[:], in_=z1[:],
                                             shift=b1s[:],
                                             bound=float(np.pi),
                                             period=float(2 * np.pi))
                    h1 = actp.tile([P, BLK], f16, tag="h1")
                    nc.scalar.activation(out=h1[:], in_=t1[:], func=Act.Sin)
                    # seed the bank with b2 (K=1 matmul), accumulate w2@h1
                    nc.tensor.matmul(o2[:, po:po + BLK], b2c[:], ones[:],
                                     start=True, stop=False)
                    nc.tensor.matmul(o2[:, po:po + BLK], w2t[:], h1[:],
                                     start=False, stop=True)
                    if b % 4 == 3:
                        # plain PSUM evac on ACT
                        nc.scalar.activation(
                            out=oacc[:, lo - 3 * BLK:lo + BLK],
                            in_=o2[:], func=Act.Copy, bias=0.0, scale=1.0)
                nc.sync.dma_start(out=out_d[:, k * CHUNK:(k + 1) * CHUNK],
                                  in_=oacc[:])

    nc.compile()
    return nc


def _host_prep(inputs):
    coords = np.ascontiguousarray(inputs["coords"], dtype=np.float32)
    emb0 = np.asarray(inputs["emb0"], dtype=np.float32)
    emb1 = np.asarray(inputs["emb1"], dtype=np.float32)
    w0 = np.asarray(inputs["w0"], dtype=np.float32)
    b0 = np.asarray(inputs["b0"], dtype=np.float32)
    w1 = np.asarray(inputs["w1"], dtype=np.float32)
    b1 = np.asarray(inputs["b1"], dtype=np.float32)
    w2 = np.asarray(inputs["w2"], dtype=np.float32)
    b2 = np.asarray(inputs["b2"], dtype=np.float32)

    def upsample(emb):
        i = np.arange(RES - 1)
        w = (np.arange(UPS, dtype=np.float64) / UPS).astype(np.float32)
        t = (1.0 - w)[None, :, None] * emb[i][:, None, :] \
            + w[None, :, None] * emb[i + 1][:, None, :]
        t = t.reshape(NROWS, ED)
        pad = np.zeros((NROWS_PAD - NROWS, ED), np.float32)
        return np.concatenate([t, pad], 0)

    u = np.concatenate([upsample(emb0), upsample(emb1)], 0)  # [2*PAD, ED]
    if FP8:
        import ml_dtypes
        up01 = np.ascontiguousarray(
            u.astype(ml_dtypes.float8_e4m3fn)).view(np.int16)
        # token layout: partition p holds dims (2p, 2p+1)
        w0t = np.ascontiguousarray(
            w0.T.reshape(P, 2, HID)).reshape(P, 2 * HID).astype(np.float16)
    else:
        up01 = u.astype(np.float16)
        # transpose layout: partition p holds dims (p, 128+p)
        w0t = np.ascontiguousarray(
            w0.T.reshape(2, P, HID).transpose(1, 0, 2)
        ).reshape(P, 2 * HID).astype(np.float16)

    w1t = np.ascontiguousarray(w1.T * W0_FREQ).astype(np.float16)
    w2t = np.ascontiguousarray(w2.T).astype(np.float16)
    b0s = (W0_FREQ * b0).reshape(P, 1).astype(np.float32)
    b1s = (W0_FREQ * b1).reshape(P, 1).astype(np.float32)
    b2c = b2.reshape(1, NOUT).astype(np.float16)

    shared = dict(up01=up01, w0t=w0t, w1t=w1t, w2t=w2t,
                  b0s=b0s, b1s=b1s, b2c=b2c)
    in_maps = []
    for c in range(NCORES):
        shard = np.ascontiguousarray(coords[c * B:(c + 1) * B])
        in_maps.append(dict(coords=shard, **shared))
    return in_maps


last_results = None


def kernel(**inputs):
    global last_results
    from concourse.bass_utils import run_bass_kernel_spmd

    if "nc" not in _cache:
        _cache["nc"] = _build_nc()
    nc = _cache["nc"]

    in_maps = _host_prep(inputs)
    trace = bool(int(os.environ.get("KERNEL_TRACE", "0")))
    res = run_bass_kernel_spmd(nc, in_maps, core_ids=list(range(NCORES)),
                               trace=trace)
    last_results = res

    outs = []
    for c in range(NCORES):
        dev = res.results[c]["out"]          # [3, B], cols in slot order
        # slot m = k*CHUNK + f*16 + q  ->  point o = q*FPQ + k*256 + f
        dev = dev.reshape(NOUT, N_CHUNKS, CHUNK // 16, 16)
        dev = dev.transpose(0, 3, 1, 2).reshape(NOUT, B)
        outs.append(dev.T)
    return np.ascontiguousarray(
        np.concatenate(outs, 0).astype(np.float32))


# revision 7
# speedup vs baseline: 4.8823x; 1.0089x over previous
"""Trainium2 Bass kernel for nn_AxisNetwork (embedding_lookup + sine MLP).

Math per point (x, y):
    e = lerp(emb0, x) * lerp(emb1, y)          # [256]
    h = sin(30*(e @ w0.T + b0))                # [128]
    h = sin(30*(h @ w1.T + b1))                # [128]
    out = h @ w2.T + b2                        # [3]

Device strategy (pure data parallel over 8 cores, B = N/8 points each):
  * Per-axis linear interpolation is replaced by a lookup into a U=32x
    upsampled snap table (host-precomputed from emb0/emb1). Both axis
    tables live in ONE fp8 DRAM tensor (y rows offset by NROWS_PAD);
    fp8 rows are moved as 128 int16 tokens per row.
  * Index prep stays on-chip: coords land on 16 partitions in the
    gather's wrap-16 slot order, DVE does clip/affine in place and
    converts straight into the wrapped int16 index tile, which is then
    replicated to all 8 partition groups with 7 SBUF->SBUF DMAs.
  * Per 4096-point chunk: two dma_gathers (x rows, y rows; SWDGE,
    transpose=True) land fp8 channel-pairs-on-partition; DVE forms
    ee = e0*e1 in fp16; PE runs the three matmuls with points as
    columns; ACT applies sin via its scale/bias fold; layer 2 keeps w2
    stationary producing [3, n] so its matmul is one N=512 stream.
  * Output streams out as [3, B]; host de-permutes and transposes.
"""

import os

import numpy as np

N_FULL = 1 << 20
NCORES = 8
B = int(os.environ.get("KERNEL_B", N_FULL // NCORES))  # points per core
RES = 512
ED = 256
HID = 128
NOUT = 3
W0_FREQ = 30.0

FP8 = bool(int(os.environ.get("KERNEL_FP8", "1")))
UPS = 32                  # upsample factor for the snap tables
NROWS = (RES - 1) * UPS   # valid rows per axis
NROWS_PAD = 16384

CHUNK = 4096              # points per chunk (one x-gather + one y-gather)
N_CHUNKS = B // CHUNK
BLK = 512                 # points per MLP block
BLKS = CHUNK // BLK

P = 128
FPQ = B // 16             # points per wrap partition (8192)

_cache = {}


def _build_nc():
    import concourse.bacc as bacc
    import concourse.bass as bass
    import concourse.mybir as mybir
    import concourse.tile as tile
    from concourse import library_config

    f32 = mybir.dt.float32
    f16 = mybir.dt.float16
    i16 = mybir.dt.int16
    Alu = mybir.AluOpType
    Act = mybir.ActivationFunctionType

    # fp8 rows move as int16 tokens (2 fp8 values per token)
    tdt = i16 if FP8 else f16
    ESZ = 128 if FP8 else ED          # gather elem_size in tdt units

    nc = bacc.Bacc("TRN2", target_bir_lowering=False, debug=False,
                   num_devices=NCORES, num_swdge_queues=4)

    coords_d = nc.dram_tensor("coords", [B, 2], f32, kind="ExternalInput")
    up_d = nc.dram_tensor("up01", [2 * NROWS_PAD, ESZ], tdt,
                          kind="ExternalInput")
    w0t_d = nc.dram_tensor("w0t", [P, 2 * HID], f16, kind="ExternalInput")
    w1t_d = nc.dram_tensor("w1t", [HID, HID], f16, kind="ExternalInput")
    w2t_d = nc.dram_tensor("w2t", [HID, NOUT], f16, kind="ExternalInput")
    b0s_d = nc.dram_tensor("b0s", [P, 1], f32, kind="ExternalInput")
    b1s_d = nc.dram_tensor("b1s", [P, 1], f32, kind="ExternalInput")
    b2c_d = nc.dram_tensor("b2c", [1, NOUT], f16, kind="ExternalInput")
    out_d = nc.dram_tensor("out", [NOUT, B], f32, kind="ExternalOutput")

    AFF = 255.5 * UPS       # (0.5c+0.5)*511*UPS == c*AFF + AFF

    with tile.TileContext(nc) as tc:
        with (
            tc.tile_pool(name="const", bufs=1) as cpool,
            tc.tile_pool(name="prep", bufs=1) as prep,
            tc.tile_pool(name="idx", bufs=1) as idxp,
            tc.tile_pool(name="gath", bufs=2) as gath,
            tc.tile_pool(name="eep", bufs=1) as eep,
            tc.tile_pool(name="act", bufs=2) as actp,
            tc.tile_pool(name="oac", bufs=2) as oacp,
            tc.tile_pool(name="psA", bufs=2, space="PSUM") as psA,
            tc.tile_pool(name="psB", bufs=2, space="PSUM") as psB,
            tc.tile_pool(name="psC", bufs=1, space="PSUM") as psC,
        ):
            nc.gpsimd.load_library(library_config.mlp)

            # ---- constants / weights ----
            w0t = cpool.tile([P, 2, HID], f16)
            nc.sync.dma_start(out=w0t[:].rearrange("p c h -> p (c h)"),
                              in_=w0t_d[:])
            w1t = cpool.tile([HID, HID], f16)
            nc.sync.dma_start(out=w1t[:], in_=w1t_d[:])
            w2t = cpool.tile([HID, NOUT], f16)
            nc.sync.dma_start(out=w2t[:], in_=w2t_d[:])
            b0s = cpool.tile([P, 1], f32)
            nc.sync.dma_start(out=b0s[:], in_=b0s_d[:])
            b1s = cpool.tile([P, 1], f32)
            nc.sync.dma_start(out=b1s[:], in_=b1s_d[:])
            b2c = cpool.tile([1, NOUT], f16)
            nc.sync.dma_start(out=b2c[:], in_=b2c_d[:])
            ones = cpool.tile([1, BLK], f16)
            nc.vector.memset(ones[:], 1.0)

            # ---- index prep (all on-chip) ----
            # coords in wrap-16 slot order: partition q holds points
            # [q*8192, (q+1)*8192) as (x, y) pairs, 64 KB contiguous.
            ctile = prep.tile([16, FPQ, 2], f32)
            nc.sync.dma_start(
                out=ctile[:], in_=coords_d[:].rearrange("(q f) a -> q f a",
                                                        q=16))
            cfl = ctile[:].rearrange("q f a -> q (f a)")
            # clip to [-1, 0.999] in place (as the reference does)
            nc.vector.tensor_scalar(out=cfl, in0=cfl, scalar1=0.999,
                                    scalar2=-1.0, op0=Alu.min, op1=Alu.max)
            # affine to upsampled-row coords in place; y rows offset by
            # NROWS_PAD into the combined table
            cx = ctile[:, :, 0]
            cy = ctile[:, :, 1]
            nc.vector.tensor_scalar(out=cx, in0=cx, scalar1=AFF,
                                    scalar2=AFF, op0=Alu.mult, op1=Alu.add)
            nc.vector.tensor_scalar(out=cy, in0=cy, scalar1=AFF,
                                    scalar2=AFF + NROWS_PAD,
                                    op0=Alu.mult, op1=Alu.add)
            # wrapped index tile: per chunk k the layout is
            # [x idx of 256 points | y idx of 256 points] per partition.
            ixy = idxp.tile([P, N_CHUNKS, 2, CHUNK // 16], i16)
            iv = ixy[0:16]
            # f32 -> i16 converts (round-to-nearest) into strided views
            nc.vector.tensor_copy(
                out=iv[:, :, 0, :],
                in_=cx.rearrange("q (k j) -> q k j", j=CHUNK // 16))
            nc.vector.tensor_copy(
                out=iv[:, :, 1, :],
                in_=cy.rearrange("q (k j) -> q k j", j=CHUNK // 16))
            # replicate to the other 7 partition groups
            for g in range(1, 8):
                nc.sync.dma_start(
                    out=ixy[16 * g:16 * (g + 1)].rearrange(
                        "q k c j -> q (k c j)"),
                    in_=ixy[0:16].rearrange("q k c j -> q (k c j)"))

            # ---- main pipeline ----
            for k in range(N_CHUNKS):
                if FP8:
                    e01 = gath.tile([P, 1, 2 * CHUNK], i16, tag="e01")
                else:
                    e01 = gath.tile([P, 2, 2 * CHUNK], f16, tag="e01")
                for a in range(2):
                    nc.gpsimd.dma_gather(
                        e01[:, :, a * CHUNK:(a + 1) * CHUNK],
                        up_d[:],
                        ixy[:, k, a, :],
                        num_idxs=CHUNK, num_idxs_reg=CHUNK, elem_size=ESZ,
                        transpose=True, single_packet=False,
                        queue_num=(2 * k + a) % 4)

                if FP8:
                    ef = e01[:].rearrange("p c n -> p (c n)").bitcast(
                        mybir.dt.float8e4)            # [P, 4*CHUNK] bytes
                    ex = ef[:, 0:2 * CHUNK]
                    ey = ef[:, 2 * CHUNK:4 * CHUNK]
                    ee = eep.tile([P, 2 * CHUNK], f16, tag="ee")
                    nc.vector.tensor_tensor(out=ee[:], in0=ex, in1=ey,
                                            op=Alu.mult)
                    # ee[p, 2n+c] = e[d=2p+c, n]
                    rhs = [ee[:].rearrange("p (n c) -> p c n", c=2)[:, c, :]
                           for c in range(2)]
                else:
                    ee = eep.tile([P, 2, CHUNK], f16, tag="ee")
                    nc.vector.tensor_tensor(
                        out=ee[:],
                        in0=e01[:, :, 0:CHUNK],
                        in1=e01[:, :, CHUNK:2 * CHUNK],
                        op=Alu.mult)
                    # ee[p, c, n] = e[d=c*128+p, n]
                    rhs = [ee[:, c, :] for c in range(2)]

                oacc = oacp.tile([NOUT, CHUNK], f32, tag="oacc")
                o2 = None
                for b in range(BLKS):
                    lo = b * BLK
                    if b % 4 == 0:
                        o2 = psC.tile([NOUT, 4 * BLK], f32, tag="o2")
                    po = (b % 4) * BLK
                    z0 = psA.tile([P, BLK], f32, tag="z0")
                    nc.tensor.matmul(z0[:], w0t[:, 0, :],
                                     rhs[0][:, lo:lo + BLK],
                                     start=True, stop=False)
                    nc.tensor.matmul(z0[:], w0t[:, 1, :],
                                     rhs[1][:, lo:lo + BLK],
                                     start=False, stop=True)
                    h0 = actp.tile([P, BLK], f16, tag="h0")
                    nc.scalar.activation(out=h0[:], in_=z0[:], func=Act.Sin,
                                         bias=b0s[:], scale=W0_FREQ)
                    z1 = psB.tile([P, BLK], f32, tag="z1")
                    nc.tensor.matmul(z1[:], w1t[:], h0[:],
                                     start=True, stop=True)
                    # ACT's Sin spline covers [-pi, pi]; w1t is pre-scaled
                    # by 30 so wrap 30*z1 + 30*b1 back into range first
                    t1 = actp.tile([P, BLK], f32, tag="t1")
                    nc.vector.add_range_wrap(out=t1[:], in_=z1[:],
                                             shift=b1s[:],
                                             bound=float(np.pi),
                                             period=float(2 * np.pi))
                    h1 = actp.tile([P, BLK], f16, tag="h1")
                    nc.scalar.activation(out=h1[:], in_=t1[:], func=Act.Sin)
                    # seed the bank with b2 (K=1 matmul), accumulate w2@h1
                    nc.tensor.matmul(o2[:, po:po + BLK], b2c[:], ones[:],
                                     start=True, stop=False)
                    nc.tensor.matmul(o2[:, po:po + BLK], w2t[:], h1[:],
                                     start=False, stop=True)
                    if b % 4 == 3:
                        # plain PSUM evac on DVE (exact f32 copy)
                        nc.vector.tensor_copy(
                            out=oacc[:, lo - 3 * BLK:lo + BLK], in_=o2[:])
                nc.sync.dma_start(out=out_d[:, k * CHUNK:(k + 1) * CHUNK],
                                  in_=oacc[:])

    nc.compile()
    return nc


def _host_prep(inputs):
    coords = np.ascontiguousarray(inputs["coords"], dtype=np.float32)
    emb0 = np.asarray(inputs["emb0"], dtype=np.float32)
    emb1 = np.asarray(inputs["emb1"], dtype=np.float32)
    w0 = np.asarray(inputs["w0"], dtype=np.float32)
    b0 = np.asarray(inputs["b0"], dtype=np.float32)
    w1 = np.asarray(inputs["w1"], dtype=np.float32)
    b1 = np.asarray(inputs["b1"], dtype=np.float32)
    w2 = np.asarray(inputs["w2"], dtype=np.float32)
    b2 = np.asarray(inputs["b2"], dtype=np.float32)

    def upsample(emb):
        i = np.arange(RES - 1)
        w = (np.arange(UPS, dtype=np.float64) / UPS).astype(np.float32)
        t = (1.0 - w)[None, :, None] * emb[i][:, None, :] \
            + w[None, :, None] * emb[i + 1][:, None, :]
        t = t.reshape(NROWS, ED)
        pad = np.zeros((NROWS_PAD - NROWS, ED), np.float32)
        return np.concatenate([t, pad], 0)

    u = np.concatenate([upsample(emb0), upsample(emb1)], 0)  # [2*PAD, ED]
    if FP8:
        import ml_dtypes
        up01 = np.ascontiguousarray(
            u.astype(ml_dtypes.float8_e4m3fn)).view(np.int16)
        # token layout: partition p holds dims (2p, 2p+1)
        w0t = np.ascontiguousarray(
            w0.T.reshape(P, 2, HID)).reshape(P, 2 * HID).astype(np.float16)
    else:
        up01 = u.astype(np.float16)
        # transpose layout: partition p holds dims (p, 128+p)
        w0t = np.ascontiguousarray(
            w0.T.reshape(2, P, HID).transpose(1, 0, 2)
        ).reshape(P, 2 * HID).astype(np.float16)

    w1t = np.ascontiguousarray(w1.T * W0_FREQ).astype(np.float16)
    w2t = np.ascontiguousarray(w2.T).astype(np.float16)
    b0s = (W0_FREQ * b0).reshape(P, 1).astype(np.float32)
    b1s = (W0_FREQ * b1).reshape(P, 1).astype(np.float32)
    b2c = b2.reshape(1, NOUT).astype(np.float16)

    shared = dict(up01=up01, w0t=w0t, w1t=w1t, w2t=w2t,
                  b0s=b0s, b1s=b1s, b2c=b2c)
    in_maps = []
    for c in range(NCORES):
        shard = np.ascontiguousarray(coords[c * B:(c + 1) * B])
        in_maps.append(dict(coords=shard, **shared))
    return in_maps


last_results = None


def kernel(**inputs):
    global last_results
    from concourse.bass_utils import run_bass_kernel_spmd

    if "nc" not in _cache:
        _cache["nc"] = _build_nc()
    nc = _cache["nc"]

    in_maps = _host_prep(inputs)
    trace = bool(int(os.environ.get("KERNEL_TRACE", "0")))
    res = run_bass_kernel_spmd(nc, in_maps, core_ids=list(range(NCORES)),
                               trace=trace)
    last_results = res

    outs = []
    for c in range(NCORES):
        dev = res.results[c]["out"]          # [3, B], cols in slot order
        # slot m = k*CHUNK + f*16 + q  ->  point o = q*FPQ + k*256 + f
        dev = dev.reshape(NOUT, N_CHUNKS, CHUNK // 16, 16)
        dev = dev.transpose(0, 3, 1, 2).reshape(NOUT, B)
        outs.append(dev.T)
    return np.ascontiguousarray(
        np.concatenate(outs, 0).astype(np.float32))
